# revision 29
# baseline (speedup 1.0000x reference)
"""Trainium2 Bass kernel for nn_ColorRestoration — transposed/PE version.

Math (per image row, W = 3072, w_ceil = 14, RGB_IDX = (3, 7, 10)):
    u_c[t]   = x[t + idx_c] * z[t]                (x zero-padded on the right)
    y[c, p]  = ms14(u_c)[p] / ms14(z)[p]          (backward moving sums, width 14)
    rgb[c,p] = z[p - idx_c]                       (z zero-padded on the left)

Layout: host transposes inputs so W sits on SBUF partitions and H (2048
image rows) on the free dim.  W shards across the 8 cores (384 wpos each
plus halo); each core runs 4 shingled blocks of 128 input wpos -> 115
output wpos.  Every width-14 moving sum is then a banded matmul
(B[k, m] = 1 iff m <= k <= m+13) on the otherwise-idle TensorEngine with
f32 PSUM accumulation, which removes the ~52us of DVE
tensor_tensor_scan work that bounded the row-major kernel.

The channel shift sits on the x side (u_c reads x[t+idx_c]), so the x
tiles are simply loaded three extra times at shifted DRAM row offsets
(a partition shift in transposed layout is just a row offset in DRAM),
one band matrix serves all four signals, and rgb outputs are
partition-windowed DMA reads of the single z SBUF copy.

Engine split per core:
  - PE: 16 banded matmuls (4 signals x 4 blocks, 4x512-row chunks each)
  - DVE: 12 u-products + 12 normalize multiplies, all bf16 2x mode
  - ACT: 4 reciprocals of ms14(z) + 12 PSUM->SBUF bf16 evacuations
  - DMA: no SBUF->SBUF traffic at all
"""

import sys

sys.path.insert(0, "/opt/trn_rl_repo")

import ml_dtypes
import numpy as np

import concourse.bass as bass
import concourse.mybir as mybir
import concourse.tile as tile
from concourse import bass_utils

F32 = mybir.dt.float32
BF16 = mybir.dt.bfloat16
FP8 = mybir.dt.float8e4
OP = mybir.AluOpType
AF = mybir.ActivationFunctionType

G = 14                     # moving-sum width
RGB_IDX = (3, 7, 10)
N_CORES = 8
H, W = 2048, 3072
WS = W // N_CORES          # 384 wpos per core
V = 115                    # valid outputs per 128-wide block (128 - 13)
NB = 4                     # blocks per core (3*115 + 39 = 384)
HALO_LO = 13               # left halo: window reaches 13 back
NROW = 496                 # DRAM rows per core: wpos [cs-13, cs+483)
ROWS = H                   # free dim = 2048 image rows


def split_waits(nc, maxw=1):
    """Split multi-wait instructions into single-wait NOPs (walrus limit)."""
    uid = 0
    for f in nc.m.functions:
        for b in f.blocks:
            out, changed = [], False
            for ins in b.instructions:
                si = ins.sync_info
                if si is not None and len(si.on_wait) > maxw:
                    waits = list(si.on_wait)
                    keep, rest = waits[-maxw:], waits[:-maxw]
                    for i in range(0, len(rest), maxw):
                        nop = mybir.InstNoOp(name=f"splitw-{uid}", engine=ins.engine)
                        uid += 1
                        nop.sync_info = mybir.SyncInfo(
                            on_wait=rest[i : i + maxw], on_update=[]
                        )
                        nc.register_instruction(nop)
                        out.append(nop)
                    ins.sync_info = mybir.SyncInfo(
                        on_wait=keep, on_update=list(si.on_update)
                    )
                    changed = True
                out.append(ins)
            if changed:
                b.instructions = out


def act_op(nc, func, out, in_):
    """Raw InstActivation on the ACT engine (Reciprocal / Copy)."""
    eng = nc.scalar
    ins = [eng.lower_ap(in_)]
    for arg in (0.0, 1.0, 0.0):  # bias, scale, alpha
        ins.append(mybir.ImmediateValue(dtype=mybir.dt.float32, value=arg))
    return eng.add_instruction(
        mybir.InstActivation(
            name=nc.get_next_instruction_name(),
            func=func,
            ins=ins,
            outs=[eng.lower_ap(out)],
        )
    )


def make_band():
    """lhsT band matrix [128, V] bf16: B[k, m] = 1 iff m <= k <= m + 13
    (out wpos = block_input_start + 13 + m)."""
    k = np.arange(128)[:, None]
    m = np.arange(V)[None, :]
    b = ((k >= m) & (k <= m + 13)).astype(np.float32)
    return b.astype(ml_dtypes.bfloat16)


def build_nc():
    """Per-core program: xt,zt bf16 [NROW, ROWS] -> yt,rgbt bf16 [3, WS, ROWS].

    DRAM row r of xt/zt holds wpos (cs - 13 + r) of the transposed image.
    Block b: input wpos start S_b = cs - 13 + 115*b (DRAM row 115*b);
    outputs y at wpos [cs + 115*b, +115) (block 3: 39 valid).
    """
    nc = bass.Bass("TRN2", debug=False)
    xt = nc.dram_tensor("xt", [NROW, ROWS], BF16, kind="ExternalInput")
    zt = nc.dram_tensor("zt", [NROW, ROWS], BF16, kind="ExternalInput")
    # fp8 copy of the z mask (0/1 exact in e4m3): rgb outputs are pure
    # DRAM->DRAM shifts of z, so doing them in fp8 halves their HBM cost
    zt8 = nc.dram_tensor("zt8", [NROW, ROWS], FP8, kind="ExternalInput")
    band = nc.dram_tensor("band", [128, V], BF16, kind="ExternalInput")
    yt = nc.dram_tensor("yt", [3, WS, ROWS], BF16, kind="ExternalOutput")
    rgbt = nc.dram_tensor("rgbt", [3, WS, ROWS], FP8, kind="ExternalOutput")

    CH = 512  # matmul moving-dim chunk (HW limit)
    NCH = ROWS // CH

    with tile.TileContext(nc) as tc:
        with tc.tile_pool(name="pool", bufs=1) as pool, tc.psum_pool(
            name="psum", bufs=1
        ) as ppool:
            zb, xs = {}, {}
            # z0 loads first (before band), in column halves: if dependency
            # tracking is region-granular the first z-matmul starts as soon
            # as cols [0, 1024) land
            zb[0] = pool.tile([128, ROWS], BF16, name="z0", tag="z0")
            nc.sync.dma_start(zb[0][:, 0 : ROWS // 2], zt[0:128, 0 : ROWS // 2])
            bnd = pool.tile([128, V], BF16, name="band", tag="band")
            nc.sync.dma_start(bnd[:, :], band[:, :])
            nc.sync.dma_start(zb[0][:, ROWS // 2 :], zt[0:128, ROWS // 2 :])
            for b in range(1, NB):
                zb[b] = pool.tile([128, ROWS], BF16, name=f"z{b}", tag=f"z{b}")
                nc.sync.dma_start(zb[b][:, :], zt[115 * b : 115 * b + 128, :])
            for b in range(NB):
                for c, idx in enumerate(RGB_IDX):
                    t = pool.tile([128, ROWS], BF16, name=f"x{c}{b}", tag=f"x{c}{b}")
                    xs[c, b] = t
                    r0 = 115 * b + idx
                    # issue on the GpSimd SWDGE: descriptor generation
                    # (~0.65us per 128-partition DMA) would otherwise
                    # serialize on the Sync sequencer and delay the z loads
                    nc.gpsimd.dma_start(t[:, :], xt[r0 : r0 + 128, :])

            # rgb out: whole channel = one contiguous DRAM->DRAM copy of zt
            # rows [13-idx, 13-idx+WS).  Issued on the GpSimd stream AFTER
            # the x loads so its 9MB of queue traffic doesn't delay them.
            for c, idx in enumerate(RGB_IDX):
                r0 = 13 - idx
                nc.gpsimd.dma_start(rgbt[c, :, :], zt8[r0 : r0 + WS, :])

            # --- per block: z matmul -> recip, then channels (interleaved
            # so the channel pipeline starts after the first z block).
            # PSUM tiles are half-width [V, 1024] (2 banks, so 4 live slots)
            # and every drain works at half-tile grain: finer pipelining of
            # PE fill / ACT evacuation / DVE normalize.
            HH = ROWS // 2  # 1024: psum tile width, 2 matmul chunks each
            nps = 0  # psum slot rotation: 4 slots x 2 banks = all 8 banks
            rcp = {}

            def z_phase(b):
                rcp[b] = pool.tile([V, ROWS], BF16, name=f"rcp{b}", tag=f"rcp{b}")
                for h in range(2):
                    # z tiles own psum slots 0-1; channels rotate on 2-3, so
                    # the next block's z matmuls never wait on channel drains
                    pz = ppool.tile(
                        [V, HH], F32, name=f"pz{b}{h}", tag=f"psz{h}"
                    )
                    for j in range(2):
                        o = j * CH
                        nc.tensor.matmul(
                            pz[:, o : o + CH],
                            bnd[:, :],
                            zb[b][:, h * HH + o : h * HH + o + CH],
                            start=True,
                            stop=True,
                        )
                    act_op(
                        nc, AF.Reciprocal, rcp[b][:, h * HH : (h + 1) * HH], pz[:, :]
                    )

            def ch_phase(b):
                nonlocal nps
                nv = min(V, WS - 115 * b)
                for c in range(3):
                    i = 3 * b + c
                    u = pool.tile(
                        [128, ROWS], BF16, name=f"u{c}{b}", tag=f"u{i % 4}"
                    )
                    nc.vector.tensor_tensor(
                        u[:, :], xs[c, b][:, :], zb[b][:, :], op=OP.mult
                    )
                    yb = pool.tile(
                        [V, ROWS], BF16, name=f"y{c}{b}", tag=f"y{i % 4}"
                    )
                    for h in range(2):
                        pc = ppool.tile(
                            [V, HH], F32, name=f"pc{c}{b}{h}", tag=f"psc{nps % 2}"
                        )
                        nps += 1
                        for j in range(2):
                            o = j * CH
                            nc.tensor.matmul(
                                pc[:, o : o + CH],
                                bnd[:, :],
                                u[:, h * HH + o : h * HH + o + CH],
                                start=True,
                                stop=True,
                            )
                        ms = pool.tile(
                            [V, HH], BF16, name=f"ms{c}{b}{h}", tag=f"ms{i % 4}{h}"
                        )
                        act_op(nc, AF.Copy, ms[0:nv, :], pc[0:nv, :])
                        nc.vector.tensor_tensor(
                            yb[0:nv, h * HH : (h + 1) * HH],
                            ms[0:nv, :],
                            rcp[b][0:nv, h * HH : (h + 1) * HH],
                            op=OP.mult,
                        )
                    nc.gpsimd.dma_start(
                        yt[c, 115 * b : 115 * b + nv, :], yb[0:nv, :]
                    )

            for b in range(NB):
                z_phase(b)
                ch_phase(b)

    split_waits(nc, maxw=1)
    return nc


_NC_CACHE = {}


def _get_nc():
    if "nc" not in _NC_CACHE:
        _NC_CACHE["nc"] = build_nc()
    return _NC_CACHE["nc"]


def _prep(x2, z2):
    """Full [H, W] f32 arrays -> per-core transposed bf16 inputs."""
    pad_lo = HALO_LO
    xtg = np.zeros((W + pad_lo + NROW, ROWS), np.float32)
    ztg = np.zeros_like(xtg)
    xtg[pad_lo : pad_lo + W] = x2.T
    ztg[pad_lo : pad_lo + W] = z2.T
    xtg = xtg.astype(ml_dtypes.bfloat16)
    ztg8 = ztg.astype(ml_dtypes.float8_e4m3)
    ztg = ztg.astype(ml_dtypes.bfloat16)
    band = make_band()
    maps = []
    for i in range(N_CORES):
        cs = i * WS
        maps.append(
            {
                "xt": np.ascontiguousarray(xtg[cs : cs + NROW]),
                "zt": np.ascontiguousarray(ztg[cs : cs + NROW]),
                "zt8": np.ascontiguousarray(ztg8[cs : cs + NROW]),
                "band": band,
            }
        )
    return maps


def run_sharded(x2, z2, trace=False, **kw):
    """x2, z2: [H, W] f32.  Returns (y, rgb) [3, H, W] f32 (+ results)."""
    nc = _get_nc()
    in_maps = _prep(x2, z2)
    res = bass_utils.run_bass_kernel_spmd(
        nc, in_maps, list(range(N_CORES)), trace=trace, **kw
    )
    yt = np.concatenate(
        [np.asarray(res.results[i]["yt"]) for i in range(N_CORES)], axis=1
    )  # [3, W, ROWS]
    rt = np.concatenate(
        [np.asarray(res.results[i]["rgbt"]) for i in range(N_CORES)], axis=1
    )
    yf = np.moveaxis(yt, 1, 2).astype(np.float32)  # [3, H, W]
    rf = np.moveaxis(rt, 1, 2).astype(np.float32)
    return yf, rf, res


def kernel(x, z):
    x2 = np.asarray(x, dtype=np.float32).reshape(H, W)
    z2 = np.asarray(z, dtype=np.float32).reshape(H, W)
    yf, rf, _ = run_sharded(x2, z2)
    return yf.reshape(1, 3, H, W), rf.reshape(1, 3, H, W)
